# revision 6
# baseline (speedup 1.0000x reference)
"""Trainium2 Bass kernel for nn_AMXReversibleLayer.

Reference computation (RevNet-style additive coupling):
    x1, x2 = split(x, 2, axis=-1)      # x: [B, S, 2D] f32, each [B, S, D]
    y = concat([x1, x2 + x1 @ W], -1)  # W: [D, D] f32

Strategy: pure data-parallel. x [8, 32768, 256] is sharded along batch —
one batch element (32768 tokens) per NeuronCore, W replicated. No
collectives. The kernel is memory-bound: per core 32 MB in + 32 MB out.

Per-core kernel (Tile framework):
  - Tokens live on SBUF partitions (128/tile), the 256 features on the
    free axis, so DMAs move 1 KB-contiguous runs per token.
  - TensorE needs the contraction dim (d) on partitions, so each
    128-token x1 tile is transposed on the PE array (matmul vs identity)
    into PSUM, copied back to SBUF, then matmul'd against W.
  - h is added in-place into the x2 columns of the input tile and the
    whole tile goes back out with one DMA.

Constraint that shapes the engine assignment: an f32 matmul lowers to
LDWEIGHTS+MATMULT and the LW slot accepts only ONE sync-wait command
(walrus "Too many sync wait commands" otherwise). So every PE
instruction must depend on a single semaphore: ALL elementwise work
(const prep, x1 staging, PSUM->SBUF copies, adds) runs on the
VectorEngine, and PE never reads DMA'd data directly (x1 is staged
through a DVE copy first).
"""

import numpy as np

import concourse.bass as bass
import concourse.mybir as mybir
from concourse.bass_utils import run_bass_kernel_spmd
from concourse.masks import make_identity
from concourse.tile import TileContext

N_CORES = 8
B, S, TWO_D = 8, 32768, 256
D = 128
P = 128

TOKENS = (B * S) // N_CORES          # tokens per core = 32768
TILES = TOKENS // P                  # 256 tiles of 128 tokens
TILES_PER_GROUP = 16                 # 16 tiles -> 2 MB per DMA direction
NGROUPS = TILES // TILES_PER_GROUP   # 16
BUNDLE = 4                           # tiles per PSUM bank ([128, 512] f32)

_CACHE = {}


def _build_nc() -> bass.Bass:
    nc = bass.Bass()
    x = nc.dram_tensor("x", [TOKENS, TWO_D], mybir.dt.float32, kind="ExternalInput")
    w = nc.dram_tensor("weight", [D, D], mybir.dt.float32, kind="ExternalInput")
    out = nc.dram_tensor("out", [TOKENS, TWO_D], mybir.dt.float32, kind="ExternalOutput")

    # [g, p, t, d] views: token = g*(T*P) + t*P + p
    xg = x.rearrange("(g t p) d -> g p t d", t=TILES_PER_GROUP, p=P)
    og = out.rearrange("(g t p) d -> g p t d", t=TILES_PER_GROUP, p=P)

    with TileContext(nc) as tc:
        with (
            tc.tile_pool(name="const", bufs=1) as const_pool,
            tc.tile_pool(name="io", bufs=3) as io_pool,
            tc.tile_pool(name="x1s", bufs=2) as x1_pool,
            tc.tile_pool(name="xT", bufs=4) as xT_pool,
            tc.tile_pool(name="psT", bufs=4, space="PSUM") as psT_pool,
            tc.tile_pool(name="psH", bufs=4, space="PSUM") as psH_pool,
        ):
            # Constants flow through DVE so PE consumers wait on the DVE
            # semaphore only (single-wait limit on LDW).
            ident_raw = const_pool.tile([P, P], mybir.dt.float32)
            make_identity(nc, ident_raw[:])
            ident = const_pool.tile([P, P], mybir.dt.float32)
            nc.vector.tensor_copy(ident[:], ident_raw[:])
            w_raw = const_pool.tile([D, D], mybir.dt.float32)
            nc.sync.dma_start(out=w_raw[:], in_=w[:, :])
            w_sb = const_pool.tile([D, D], mybir.dt.float32)
            nc.vector.tensor_copy(w_sb[:], w_raw[:])

            for g in range(NGROUPS):
                xt = io_pool.tile([P, TILES_PER_GROUP * TWO_D], mybir.dt.float32)
                xt3 = xt[:].rearrange("p (t d) -> p t d", d=TWO_D)
                nc.sync.dma_start(out=xt3, in_=xg[g])

                # Stage x1 columns contiguously (DVE) so PE transposes
                # depend on DVE, not on the DMA lane.
                x1s = x1_pool.tile([P, TILES_PER_GROUP * D], mybir.dt.float32)
                x1s3 = x1s[:].rearrange("p (t d) -> p t d", d=D)
                nc.vector.tensor_copy(x1s3, xt3[:, :, 0:D])

                for b in range(TILES_PER_GROUP // BUNDLE):
                    pT = psT_pool.tile([P, BUNDLE * D], mybir.dt.float32)
                    for j in range(BUNDLE):
                        col = (b * BUNDLE + j) * D
                        nc.tensor.transpose(
                            pT[:, j * D:(j + 1) * D], x1s[:, col:col + D], ident[:]
                        )
                    xTs = xT_pool.tile([P, BUNDLE * D], mybir.dt.float32)
                    nc.vector.tensor_copy(xTs[:], pT[:])
                    pH = psH_pool.tile([P, BUNDLE * D], mybir.dt.float32)
                    for j in range(BUNDLE):
                        nc.tensor.matmul(
                            pH[:, j * D:(j + 1) * D],
                            lhsT=xTs[:, j * D:(j + 1) * D],
                            rhs=w_sb[:],
                            start=True,
                            stop=True,
                        )
                    x2v = xt3[:, b * BUNDLE:(b + 1) * BUNDLE, D:TWO_D]
                    pHv = pH[:].rearrange("p (t d) -> p t d", d=D)
                    nc.vector.tensor_add(x2v, pHv, x2v)

                nc.sync.dma_start(out=og[g], in_=xt3)

    _split_matmul_waits(nc)
    return nc


def _split_matmul_waits(nc: bass.Bass) -> None:
    """Several walrus ISA structs (Matmult's LDWEIGHTS uop, DVE
    TensorCopy, ...) encode only ONE sync-wait command; Tile sometimes
    emits 2+ ("Too many sync wait commands"). Hoist all but one wait
    onto standalone NoOps on the same queue right before the
    instruction — queue order makes this equivalent, and the hoisted
    waits are long-satisfied by then (they are stale WAW ticks)."""
    for blk in nc.cur_f.blocks:
        out = []
        for inst in blk.instructions:
            si = inst.sync_info
            if si is not None and si.on_wait and len(si.on_wait) > 1:
                waits = list(si.on_wait)
                for wait in waits[:-1]:
                    out.append(
                        mybir.InstNoOp(
                            name=nc.get_next_instruction_name(),
                            sync_info=mybir.SyncInfo(on_wait=[wait], on_update=[]),
                            engine=inst.engine,
                            bass_nofuse=True,
                        )
                    )
                inst.sync_info = mybir.SyncInfo(
                    on_wait=[waits[-1]], on_update=list(si.on_update or [])
                )
            out.append(inst)
        blk.instructions = out


def _get_nc() -> bass.Bass:
    if "nc" not in _CACHE:
        _CACHE["nc"] = _build_nc()
    return _CACHE["nc"]


def _in_maps(x: np.ndarray, weight: np.ndarray) -> list[dict[str, np.ndarray]]:
    x = np.ascontiguousarray(np.asarray(x, dtype=np.float32)).reshape(
        N_CORES, TOKENS, TWO_D
    )
    weight = np.ascontiguousarray(np.asarray(weight, dtype=np.float32))
    return [{"x": x[i], "weight": weight} for i in range(N_CORES)]


def kernel(x: np.ndarray, weight: np.ndarray) -> np.ndarray:
    nc = _get_nc()
    res = run_bass_kernel_spmd(nc, _in_maps(x, weight), core_ids=list(range(N_CORES)))
    out = np.stack([res.results[i]["out"] for i in range(N_CORES)], axis=0)
    return out.reshape(B, S, TWO_D)


# revision 7
# speedup vs baseline: 1.1887x; 1.1887x over previous
"""Trainium2 Bass kernel for nn_AMXReversibleLayer.

Reference computation (RevNet-style additive coupling):
    x1, x2 = split(x, 2, axis=-1)      # x: [B, S, 2D] f32, each [B, S, D]
    y = concat([x1, x2 + x1 @ W], -1)  # W: [D, D] f32

Strategy: pure data-parallel. x [8, 32768, 256] is sharded along batch —
one batch element (32768 tokens) per NeuronCore, W replicated. No
collectives. The kernel is memory-bound: per core 32 MB in + 32 MB out.

Per-core kernel (Tile framework):
  - Tokens live on SBUF partitions (128/tile), the 256 features on the
    free axis, so DMAs move 1 KB-contiguous runs per token.
  - TensorE needs the contraction dim (d) on partitions, so each
    128-token x1 tile is transposed on the PE array (matmul vs identity)
    into PSUM, copied back to SBUF, then matmul'd against W.
  - h is added in-place into the x2 columns of the input tile and the
    whole tile goes back out with one DMA.

Constraint that shapes the engine assignment: an f32 matmul lowers to
LDWEIGHTS+MATMULT and the LW slot accepts only ONE sync-wait command
(walrus "Too many sync wait commands" otherwise). So every PE
instruction must depend on a single semaphore: ALL elementwise work
(const prep, x1 staging, PSUM->SBUF copies, adds) runs on the
VectorEngine, and PE never reads DMA'd data directly (x1 is staged
through a DVE copy first).
"""

import numpy as np

import concourse.bass as bass
import concourse.mybir as mybir
from concourse.bass_utils import run_bass_kernel_spmd
from concourse.masks import make_identity
from concourse.tile import TileContext

N_CORES = 8
B, S, TWO_D = 8, 32768, 256
D = 128
P = 128

TOKENS = (B * S) // N_CORES          # tokens per core = 32768
TILES = TOKENS // P                  # 256 tiles of 128 tokens
TILES_PER_GROUP = 16                 # 16 tiles -> 2 MB per DMA direction
NGROUPS = TILES // TILES_PER_GROUP   # 16
BUNDLE = 4                           # tiles per PSUM bank ([128, 512] f32)

_CACHE = {}


def _build_nc() -> bass.Bass:
    nc = bass.Bass()
    x = nc.dram_tensor("x", [TOKENS, TWO_D], mybir.dt.float32, kind="ExternalInput")
    w = nc.dram_tensor("weight", [D, D], mybir.dt.float32, kind="ExternalInput")
    out = nc.dram_tensor("out", [TOKENS, TWO_D], mybir.dt.float32, kind="ExternalOutput")

    # [g, p, t, d] views: token = p*(NGROUPS*T) + g*T + t. Partition p
    # owns a CONTIGUOUS run of tokens, so each per-partition DMA run is
    # T*2D*4 = 16 KB contiguous (vs 1 KB with interleaved mapping) —
    # far fewer descriptors at full line rate. Compute doesn't care
    # which 128 tokens form a tile.
    xg = x.rearrange("(p g t) d -> g p t d", p=P, g=NGROUPS)
    og = out.rearrange("(p g t) d -> g p t d", p=P, g=NGROUPS)

    with TileContext(nc) as tc:
        with (
            tc.tile_pool(name="const", bufs=1) as const_pool,
            tc.tile_pool(name="io", bufs=3) as io_pool,
            tc.tile_pool(name="x1s", bufs=2) as x1_pool,
            tc.tile_pool(name="xT", bufs=4) as xT_pool,
            tc.tile_pool(name="psT", bufs=4, space="PSUM") as psT_pool,
            tc.tile_pool(name="psH", bufs=4, space="PSUM") as psH_pool,
        ):
            # Constants flow through DVE so PE consumers wait on the DVE
            # semaphore only (single-wait limit on LDW).
            ident_raw = const_pool.tile([P, P], mybir.dt.float32)
            make_identity(nc, ident_raw[:])
            ident = const_pool.tile([P, P], mybir.dt.float32)
            nc.vector.tensor_copy(ident[:], ident_raw[:])
            w_raw = const_pool.tile([D, D], mybir.dt.float32)
            nc.sync.dma_start(out=w_raw[:], in_=w[:, :])
            w_sb = const_pool.tile([D, D], mybir.dt.float32)
            nc.vector.tensor_copy(w_sb[:], w_raw[:])

            for g in range(NGROUPS):
                xt = io_pool.tile([P, TILES_PER_GROUP * TWO_D], mybir.dt.float32)
                xt3 = xt[:].rearrange("p (t d) -> p t d", d=TWO_D)
                nc.sync.dma_start(out=xt3, in_=xg[g])

                # Stage x1 columns contiguously (DVE) so PE transposes
                # depend on DVE, not on the DMA lane.
                x1s = x1_pool.tile([P, TILES_PER_GROUP * D], mybir.dt.float32)
                x1s3 = x1s[:].rearrange("p (t d) -> p t d", d=D)
                nc.vector.tensor_copy(x1s3, xt3[:, :, 0:D])

                for b in range(TILES_PER_GROUP // BUNDLE):
                    pT = psT_pool.tile([P, BUNDLE * D], mybir.dt.float32)
                    for j in range(BUNDLE):
                        col = (b * BUNDLE + j) * D
                        nc.tensor.transpose(
                            pT[:, j * D:(j + 1) * D], x1s[:, col:col + D], ident[:]
                        )
                    xTs = xT_pool.tile([P, BUNDLE * D], mybir.dt.float32)
                    nc.vector.tensor_copy(xTs[:], pT[:])
                    pH = psH_pool.tile([P, BUNDLE * D], mybir.dt.float32)
                    for j in range(BUNDLE):
                        nc.tensor.matmul(
                            pH[:, j * D:(j + 1) * D],
                            lhsT=xTs[:, j * D:(j + 1) * D],
                            rhs=w_sb[:],
                            start=True,
                            stop=True,
                        )
                    x2v = xt3[:, b * BUNDLE:(b + 1) * BUNDLE, D:TWO_D]
                    pHv = pH[:].rearrange("p (t d) -> p t d", d=D)
                    nc.vector.tensor_add(x2v, pHv, x2v)

                nc.sync.dma_start(out=og[g], in_=xt3)

    _split_matmul_waits(nc)
    return nc


def _split_matmul_waits(nc: bass.Bass) -> None:
    """Several walrus ISA structs (Matmult's LDWEIGHTS uop, DVE
    TensorCopy, ...) encode only ONE sync-wait command; Tile sometimes
    emits 2+ ("Too many sync wait commands"). Hoist all but one wait
    onto standalone NoOps on the same queue right before the
    instruction — queue order makes this equivalent, and the hoisted
    waits are long-satisfied by then (they are stale WAW ticks)."""
    for blk in nc.cur_f.blocks:
        out = []
        for inst in blk.instructions:
            si = inst.sync_info
            if si is not None and si.on_wait and len(si.on_wait) > 1:
                waits = list(si.on_wait)
                for wait in waits[:-1]:
                    out.append(
                        mybir.InstNoOp(
                            name=nc.get_next_instruction_name(),
                            sync_info=mybir.SyncInfo(on_wait=[wait], on_update=[]),
                            engine=inst.engine,
                            bass_nofuse=True,
                        )
                    )
                inst.sync_info = mybir.SyncInfo(
                    on_wait=[waits[-1]], on_update=list(si.on_update or [])
                )
            out.append(inst)
        blk.instructions = out


def _get_nc() -> bass.Bass:
    if "nc" not in _CACHE:
        _CACHE["nc"] = _build_nc()
    return _CACHE["nc"]


def _in_maps(x: np.ndarray, weight: np.ndarray) -> list[dict[str, np.ndarray]]:
    x = np.ascontiguousarray(np.asarray(x, dtype=np.float32)).reshape(
        N_CORES, TOKENS, TWO_D
    )
    weight = np.ascontiguousarray(np.asarray(weight, dtype=np.float32))
    return [{"x": x[i], "weight": weight} for i in range(N_CORES)]


def kernel(x: np.ndarray, weight: np.ndarray) -> np.ndarray:
    nc = _get_nc()
    res = run_bass_kernel_spmd(nc, _in_maps(x, weight), core_ids=list(range(N_CORES)))
    out = np.stack([res.results[i]["out"] for i in range(N_CORES)], axis=0)
    return out.reshape(B, S, TWO_D)


# revision 8
# speedup vs baseline: 1.2179x; 1.0246x over previous
"""Trainium2 Bass kernel for nn_AMXReversibleLayer.

Reference computation (RevNet-style additive coupling):
    x1, x2 = split(x, 2, axis=-1)      # x: [B, S, 2D] f32, each [B, S, D]
    y = concat([x1, x2 + x1 @ W], -1)  # W: [D, D] f32

Strategy: pure data-parallel. x [8, 32768, 256] is sharded along batch —
one batch element (32768 tokens) per NeuronCore, W replicated. No
collectives. The kernel is memory-bound: per core 32 MB in + 32 MB out.

Per-core kernel (Tile framework):
  - Tokens live on SBUF partitions (128/tile), the 256 features on the
    free axis, so DMAs move 1 KB-contiguous runs per token.
  - TensorE needs the contraction dim (d) on partitions, so each
    128-token x1 tile is transposed on the PE array (matmul vs identity)
    into PSUM, copied back to SBUF, then matmul'd against W.
  - h is added in-place into the x2 columns of the input tile and the
    whole tile goes back out with one DMA.

Constraint that shapes the engine assignment: an f32 matmul lowers to
LDWEIGHTS+MATMULT and the LW slot accepts only ONE sync-wait command
(walrus "Too many sync wait commands" otherwise). So every PE
instruction must depend on a single semaphore: ALL elementwise work
(const prep, x1 staging, PSUM->SBUF copies, adds) runs on the
VectorEngine, and PE never reads DMA'd data directly (x1 is staged
through a DVE copy first).
"""

import numpy as np

import concourse.bass as bass
import concourse.mybir as mybir
from concourse.bass_utils import run_bass_kernel_spmd
from concourse.masks import make_identity
from concourse.tile import TileContext

N_CORES = 8
B, S, TWO_D = 8, 32768, 256
D = 128
P = 128

TOKENS = (B * S) // N_CORES          # tokens per core = 32768
TILES = TOKENS // P                  # 256 tiles of 128 tokens
TILES_PER_GROUP = 8                  # 8 tiles -> 1 MB per DMA direction
NGROUPS = TILES // TILES_PER_GROUP   # 16
BUNDLE = 4                           # tiles per PSUM bank ([128, 512] f32)

_CACHE = {}


def _build_nc() -> bass.Bass:
    nc = bass.Bass()
    x = nc.dram_tensor("x", [TOKENS, TWO_D], mybir.dt.float32, kind="ExternalInput")
    w = nc.dram_tensor("weight", [D, D], mybir.dt.float32, kind="ExternalInput")
    out = nc.dram_tensor("out", [TOKENS, TWO_D], mybir.dt.float32, kind="ExternalOutput")

    # [g, p, t, d] views: token = p*(NGROUPS*T) + g*T + t. Partition p
    # owns a CONTIGUOUS run of tokens, so each per-partition DMA run is
    # T*2D*4 = 16 KB contiguous (vs 1 KB with interleaved mapping) —
    # far fewer descriptors at full line rate. Compute doesn't care
    # which 128 tokens form a tile.
    xg = x.rearrange("(p g t) d -> g p t d", p=P, g=NGROUPS)
    og = out.rearrange("(p g t) d -> g p t d", p=P, g=NGROUPS)

    with TileContext(nc) as tc:
        with (
            tc.tile_pool(name="const", bufs=1) as const_pool,
            tc.tile_pool(name="io", bufs=4) as io_pool,
            tc.tile_pool(name="x1s", bufs=3) as x1_pool,
            tc.tile_pool(name="xT", bufs=4) as xT_pool,
            tc.tile_pool(name="psT", bufs=4, space="PSUM") as psT_pool,
            tc.tile_pool(name="psH", bufs=4, space="PSUM") as psH_pool,
        ):
            # Constants flow through DVE so PE consumers wait on the DVE
            # semaphore only (single-wait limit on LDW).
            ident_raw = const_pool.tile([P, P], mybir.dt.float32)
            make_identity(nc, ident_raw[:])
            ident = const_pool.tile([P, P], mybir.dt.float32)
            nc.vector.tensor_copy(ident[:], ident_raw[:])
            w_raw = const_pool.tile([D, D], mybir.dt.float32)
            nc.sync.dma_start(out=w_raw[:], in_=w[:, :])
            w_sb = const_pool.tile([D, D], mybir.dt.float32)
            nc.vector.tensor_copy(w_sb[:], w_raw[:])

            for g in range(NGROUPS):
                xt = io_pool.tile([P, TILES_PER_GROUP * TWO_D], mybir.dt.float32)
                xt3 = xt[:].rearrange("p (t d) -> p t d", d=TWO_D)
                nc.sync.dma_start(out=xt3, in_=xg[g])

                # Stage x1 columns contiguously (DVE) so PE transposes
                # depend on DVE, not on the DMA lane.
                x1s = x1_pool.tile([P, TILES_PER_GROUP * D], mybir.dt.float32)
                x1s3 = x1s[:].rearrange("p (t d) -> p t d", d=D)
                nc.vector.tensor_copy(x1s3, xt3[:, :, 0:D])

                for b in range(TILES_PER_GROUP // BUNDLE):
                    pT = psT_pool.tile([P, BUNDLE * D], mybir.dt.float32)
                    for j in range(BUNDLE):
                        col = (b * BUNDLE + j) * D
                        nc.tensor.transpose(
                            pT[:, j * D:(j + 1) * D], x1s[:, col:col + D], ident[:]
                        )
                    xTs = xT_pool.tile([P, BUNDLE * D], mybir.dt.float32)
                    nc.vector.tensor_copy(xTs[:], pT[:])
                    pH = psH_pool.tile([P, BUNDLE * D], mybir.dt.float32)
                    for j in range(BUNDLE):
                        nc.tensor.matmul(
                            pH[:, j * D:(j + 1) * D],
                            lhsT=xTs[:, j * D:(j + 1) * D],
                            rhs=w_sb[:],
                            start=True,
                            stop=True,
                        )
                    x2v = xt3[:, b * BUNDLE:(b + 1) * BUNDLE, D:TWO_D]
                    pHv = pH[:].rearrange("p (t d) -> p t d", d=D)
                    nc.vector.tensor_add(x2v, pHv, x2v)

                nc.sync.dma_start(out=og[g], in_=xt3)

    _split_matmul_waits(nc)
    return nc


def _split_matmul_waits(nc: bass.Bass) -> None:
    """Several walrus ISA structs (Matmult's LDWEIGHTS uop, DVE
    TensorCopy, ...) encode only ONE sync-wait command; Tile sometimes
    emits 2+ ("Too many sync wait commands"). Hoist all but one wait
    onto standalone NoOps on the same queue right before the
    instruction — queue order makes this equivalent, and the hoisted
    waits are long-satisfied by then (they are stale WAW ticks)."""
    for blk in nc.cur_f.blocks:
        out = []
        for inst in blk.instructions:
            si = inst.sync_info
            if si is not None and si.on_wait and len(si.on_wait) > 1:
                waits = list(si.on_wait)
                for wait in waits[:-1]:
                    out.append(
                        mybir.InstNoOp(
                            name=nc.get_next_instruction_name(),
                            sync_info=mybir.SyncInfo(on_wait=[wait], on_update=[]),
                            engine=inst.engine,
                            bass_nofuse=True,
                        )
                    )
                inst.sync_info = mybir.SyncInfo(
                    on_wait=[waits[-1]], on_update=list(si.on_update or [])
                )
            out.append(inst)
        blk.instructions = out


def _get_nc() -> bass.Bass:
    if "nc" not in _CACHE:
        _CACHE["nc"] = _build_nc()
    return _CACHE["nc"]


def _in_maps(x: np.ndarray, weight: np.ndarray) -> list[dict[str, np.ndarray]]:
    x = np.ascontiguousarray(np.asarray(x, dtype=np.float32)).reshape(
        N_CORES, TOKENS, TWO_D
    )
    weight = np.ascontiguousarray(np.asarray(weight, dtype=np.float32))
    return [{"x": x[i], "weight": weight} for i in range(N_CORES)]


def kernel(x: np.ndarray, weight: np.ndarray) -> np.ndarray:
    nc = _get_nc()
    res = run_bass_kernel_spmd(nc, _in_maps(x, weight), core_ids=list(range(N_CORES)))
    out = np.stack([res.results[i]["out"] for i in range(N_CORES)], axis=0)
    return out.reshape(B, S, TWO_D)


# revision 11
# speedup vs baseline: 1.4016x; 1.1508x over previous
"""Trainium2 Bass kernel for nn_AMXReversibleLayer.

Reference computation (RevNet-style additive coupling):
    x1, x2 = split(x, 2, axis=-1)      # x: [B, S, 2D] f32, each [B, S, D]
    y = concat([x1, x2 + x1 @ W], -1)  # W: [D, D] f32

Strategy: pure data-parallel. x [8, 32768, 256] is sharded along batch —
one batch element (32768 tokens) per NeuronCore, W replicated. No
collectives. The kernel is memory-bound: per core 32 MB in + 32 MB out.

Per-core kernel (Tile framework):
  - Tokens live on SBUF partitions (128/tile), the 256 features on the
    free axis, so DMAs move 1 KB-contiguous runs per token.
  - TensorE needs the contraction dim (d) on partitions, so each
    128-token x1 tile is transposed on the PE array (matmul vs identity)
    into PSUM, copied back to SBUF, then matmul'd against W.
  - h is added in-place into the x2 columns of the input tile and the
    whole tile goes back out with one DMA.

Constraint that shapes the engine assignment: an f32 matmul lowers to
LDWEIGHTS+MATMULT and the LW slot accepts only ONE sync-wait command
(walrus "Too many sync wait commands" otherwise). So every PE
instruction must depend on a single semaphore: ALL elementwise work
(const prep, x1 staging, PSUM->SBUF copies, adds) runs on the
VectorEngine, and PE never reads DMA'd data directly (x1 is staged
through a DVE copy first).
"""

import numpy as np

import concourse.bass as bass
import concourse.mybir as mybir
from concourse.bass_utils import run_bass_kernel_spmd
from concourse.masks import make_identity
from concourse.tile import TileContext

N_CORES = 8
B, S, TWO_D = 8, 32768, 256
D = 128
P = 128

TOKENS = (B * S) // N_CORES          # tokens per core = 32768
TILES = TOKENS // P                  # 256 tiles of 128 tokens
TILES_PER_GROUP = 32                 # 32 tiles -> 4 MB in-DMA, 32 KB runs
NGROUPS = TILES // TILES_PER_GROUP   # 8
BUNDLE = 4                           # tiles per PSUM bank ([128, 512] f32)

_CACHE = {}


def _build_nc() -> bass.Bass:
    nc = bass.Bass()
    x = nc.dram_tensor("x", [TOKENS, TWO_D], mybir.dt.float32, kind="ExternalInput")
    w = nc.dram_tensor("weight", [D, D], mybir.dt.float32, kind="ExternalInput")
    out = nc.dram_tensor("out", [TOKENS, TWO_D], mybir.dt.float32, kind="ExternalOutput")

    # [g, p, t, d] views: token = p*(NGROUPS*T) + g*T + t. Partition p
    # owns a CONTIGUOUS run of tokens, so each per-partition DMA run is
    # T*2D*4 = 16 KB contiguous (vs 1 KB with interleaved mapping) —
    # far fewer descriptors at full line rate. Compute doesn't care
    # which 128 tokens form a tile.
    xg = x.rearrange("(p g t) d -> g p t d", p=P, g=NGROUPS)
    og = out.rearrange("(p g t) d -> g p t d", p=P, g=NGROUPS)

    with TileContext(nc) as tc:
        with (
            tc.tile_pool(name="const", bufs=1) as const_pool,
            tc.tile_pool(name="io", bufs=3) as io_pool,
            tc.tile_pool(name="xT", bufs=4) as xT_pool,
            tc.tile_pool(name="psT", bufs=4, space="PSUM") as psT_pool,
            tc.tile_pool(name="psH", bufs=4, space="PSUM") as psH_pool,
        ):
            # Constants flow through DVE so PE consumers wait on the DVE
            # semaphore only (single-wait limit on LDW).
            ident_raw = const_pool.tile([P, P], mybir.dt.float32)
            make_identity(nc, ident_raw[:])
            ident = const_pool.tile([P, P], mybir.dt.float32)
            nc.vector.tensor_copy(ident[:], ident_raw[:])
            w_raw = const_pool.tile([D, D], mybir.dt.float32)
            nc.sync.dma_start(out=w_raw[:], in_=w[:, :])
            w_sb = const_pool.tile([D, D], mybir.dt.float32)
            nc.vector.tensor_copy(w_sb[:], w_raw[:])

            half = TILES_PER_GROUP // 2
            for g in range(NGROUPS):
                xt = io_pool.tile([P, TILES_PER_GROUP * TWO_D], mybir.dt.float32)
                xt3 = xt[:].rearrange("p (t d) -> p t d", d=TWO_D)
                nc.sync.dma_start(out=xt3, in_=xg[g])

                for b in range(TILES_PER_GROUP // BUNDLE):
                    pT = psT_pool.tile([P, BUNDLE * D], mybir.dt.float32)
                    for j in range(BUNDLE):
                        col = (b * BUNDLE + j) * TWO_D
                        nc.tensor.transpose(
                            pT[:, j * D:(j + 1) * D], xt[:, col:col + D], ident[:]
                        )
                    xTs = xT_pool.tile([P, BUNDLE * D], mybir.dt.float32)
                    nc.scalar.copy(out=xTs[:], in_=pT[:])
                    pH = psH_pool.tile([P, BUNDLE * D], mybir.dt.float32)
                    for j in range(BUNDLE):
                        nc.tensor.matmul(
                            pH[:, j * D:(j + 1) * D],
                            lhsT=xTs[:, j * D:(j + 1) * D],
                            rhs=w_sb[:],
                            start=True,
                            stop=True,
                        )
                    x2v = xt3[:, b * BUNDLE:(b + 1) * BUNDLE, D:TWO_D]
                    pHv = pH[:].rearrange("p (t d) -> p t d", d=D)
                    nc.vector.tensor_add(x2v, pHv, x2v)

                    # Flush each finished half of the group so the out
                    # DMA trails the adds instead of waiting for the
                    # whole group (shorter pipeline tail).
                    tiles_done = (b + 1) * BUNDLE
                    if tiles_done % half == 0:
                        h0 = tiles_done - half
                        nc.sync.dma_start(
                            out=og[g][:, h0:tiles_done],
                            in_=xt3[:, h0:tiles_done],
                        )

    _split_matmul_waits(nc)
    return nc


def _split_matmul_waits(nc: bass.Bass) -> None:
    """Several walrus ISA structs (Matmult's LDWEIGHTS uop, DVE
    TensorCopy, ...) encode only ONE sync-wait command; Tile sometimes
    emits 2+ ("Too many sync wait commands"). Hoist all but one wait
    onto standalone NoOps on the same queue right before the
    instruction — queue order makes this equivalent, and the hoisted
    waits are long-satisfied by then (they are stale WAW ticks)."""
    for blk in nc.cur_f.blocks:
        out = []
        for inst in blk.instructions:
            si = inst.sync_info
            if si is not None and si.on_wait and len(si.on_wait) > 1:
                waits = list(si.on_wait)
                for wait in waits[:-1]:
                    out.append(
                        mybir.InstNoOp(
                            name=nc.get_next_instruction_name(),
                            sync_info=mybir.SyncInfo(on_wait=[wait], on_update=[]),
                            engine=inst.engine,
                            bass_nofuse=True,
                        )
                    )
                inst.sync_info = mybir.SyncInfo(
                    on_wait=[waits[-1]], on_update=list(si.on_update or [])
                )
            out.append(inst)
        blk.instructions = out


def _get_nc() -> bass.Bass:
    if "nc" not in _CACHE:
        _CACHE["nc"] = _build_nc()
    return _CACHE["nc"]


def _in_maps(x: np.ndarray, weight: np.ndarray) -> list[dict[str, np.ndarray]]:
    x = np.ascontiguousarray(np.asarray(x, dtype=np.float32)).reshape(
        N_CORES, TOKENS, TWO_D
    )
    weight = np.ascontiguousarray(np.asarray(weight, dtype=np.float32))
    return [{"x": x[i], "weight": weight} for i in range(N_CORES)]


def kernel(x: np.ndarray, weight: np.ndarray) -> np.ndarray:
    nc = _get_nc()
    res = run_bass_kernel_spmd(nc, _in_maps(x, weight), core_ids=list(range(N_CORES)))
    out = np.stack([res.results[i]["out"] for i in range(N_CORES)], axis=0)
    return out.reshape(B, S, TWO_D)
